# revision 37
# baseline (speedup 1.0000x reference)
"""Trainium2 Bass kernel for nn_ContinuousConvolutionBlock (gnn_message_passing).

Strategy (per sharding hint: partition points across 8 cores; each core owns its
queries' scatter-reduce and tap-GEMM; filter + dense weights replicated):

Host side (index plumbing / input marshalling only - zero FLOPs):
  - qry_idx is sorted; queries are grouped into 8-query blocks, blocks paired
    into 128-edge-slot "chunks" (two-pointer bin packing, ~3% padding).
  - Per-core per-slot payload arrays (pos[src], pos[qry], feats[src] in bf16,
    and the expanded query one-hot qexp[slot, chunk, hq, t] in bf16) are
    marshalled on host and DMA'd in dense layouts.  qexp is pure indexing
    (0/1 one-hot replicated over the 16 tap-pairs) - uploading it lets the
    DVE build the L matmul operand at 2x packed-bf16 rate.

Device side (all FLOP-bearing compute):
  - Geometry: ball->cube volume-preserving map on unscaled deltas (the map is
    homogeneous; the 2/EXTENT scale folds into the corner transform), with
    x/y lanes processed as [*, 2] pairs and reciprocal_approx_fast.
  - Corner weights via the tent identity  w4[ax] = relu(1 - |g - ax|)
    (equivalent to the (1-f, f) one-hot pair incl. boundary clipping): one
    DVE op for d4 = 37.5*m + 1.5 - ax over all 3 axes, two ACT ops for
    abs + relu -> w4 [128, W, 3, 4] bf16.
  - Scatter-reduce as factored matmul per chunk (bf16): R[slot,(ax,c)] =
    w4x (x) feats, L[slot,(hq,t)] = qexp (.) (w4z (x) w4y  broadcast);
    PE computes A^T[(ax,c),(hq,t)] = R^T @ L per chunk into PSUM.
    L/R are built in one fused DVE op per 16-chunk group; L runs in the
    2x_1P packed-bf16 mode (all operands innermost step-1).
  - PSUM->SBUF copies of A^T run on the scalar engine (ACT), overlapping the
    DVE operand builds for the next group.
  - Tap-GEMM: out^T += G_t^T @ A^T-slices over the 16 tap-pairs t (f32r).
  - Dense branch: out_dense^T = dense_w^T @ feats^T (bf16) + bias via ACT.
  Outputs are produced transposed ([64, nq]); host transposes/reorders back.
"""
import sys
import os
sys.path.insert(0, '/opt/trn_rl_repo')
import numpy as np
from ml_dtypes import bfloat16

N = 30000
CIN = 32
COUT = 64
KS = 4
EXTENT = 0.08
NCORES = 8
NBLK = N // 8  # 3750 eight-query blocks

_COMPILED = {}

# Note: walrus --enable-ldw-opt=true was tried for the LDWEIGHTS-bound
# stage-1 (218ns/chunk vs 107ns matmul) but the resulting NEFF took the
# device down (NRT_EXEC_UNIT_UNRECOVERABLE); stock flags are kept.


# ----------------------------------------------------------------------------
# Host planning
# ----------------------------------------------------------------------------
def _plan(qry_idx):
    deg = np.bincount(qry_idx, minlength=N)
    bsz = deg.reshape(NBLK, 8).sum(1)
    bstart = np.concatenate([[0], np.cumsum(bsz)]).astype(np.int64)
    per = [NBLK // NCORES + (1 if c < NBLK % NCORES else 0) for c in range(NCORES)]
    b0 = np.concatenate([[0], np.cumsum(per)]).astype(np.int64)
    plans = []
    for c in range(NCORES):
        blocks = list(range(b0[c], b0[c + 1]))
        asc = sorted(blocks, key=lambda b: bsz[b])
        chunks = []
        lo, hi = 0, len(asc) - 1
        while lo <= hi:
            if lo == hi:
                chunks.append((asc[hi], None)); break
            if bsz[asc[hi]] + bsz[asc[lo]] <= 128:
                chunks.append((asc[hi], asc[lo])); hi -= 1; lo += 1
            else:
                chunks.append((asc[hi], None)); hi -= 1
        plans.append(dict(blocks=blocks, chunks=chunks, q0=int(8 * b0[c]),
                          nq=int(8 * (b0[c + 1] - b0[c]))))
    return plans, bstart, bsz


def _pack_core(plan_c, bstart, pos, feats, qry_idx, src_idx, NCHP):
    """Build per-slot payload arrays."""
    possrc = np.zeros((128, NCHP, 3), np.float32)
    posqry = np.zeros((128, NCHP, 3), np.float32)
    fsrc = np.zeros((128, NCHP, CIN), np.float32)
    qloc = np.full((128, NCHP), -1, np.int32)
    for ci, (bA, bB) in enumerate(plan_c['chunks']):
        s = 0
        for half, b in enumerate((bA, bB)):
            if b is None:
                continue
            e0, e1 = int(bstart[b]), int(bstart[b + 1])
            n = e1 - e0
            sl = slice(s, s + n)
            possrc[sl, ci, :] = pos[src_idx[e0:e1]]
            posqry[sl, ci, :] = pos[qry_idx[e0:e1]]
            fsrc[sl, ci, :] = feats[src_idx[e0:e1]]
            qloc[sl, ci] = (qry_idx[e0:e1] - 8 * b) + 8 * half
            s += n
    # expanded one-hot: qexp[s, w, hq, t] = (qloc[s,w] == hq), any t
    oh = (qloc[:, :, None] == np.arange(16, dtype=np.int32)[None, None, :])
    qexp = np.broadcast_to(oh[:, :, :, None], (128, NCHP, 16, 16))
    qexp = np.ascontiguousarray(qexp).astype(bfloat16).reshape(128, NCHP * 256)
    return possrc, posqry, fsrc.astype(bfloat16), qexp


def _prepare(feats, pos, filt, dense_w, dense_b, src_idx, qry_idx):
    """Plan + marshal all per-core input maps. Returns (NCHP, NQ, plans, in_maps)."""
    feats = np.ascontiguousarray(np.asarray(feats, np.float32))
    pos = np.ascontiguousarray(np.asarray(pos, np.float32))
    filt = np.asarray(filt, np.float32)
    dense_w = np.asarray(dense_w, np.float32)
    dense_b = np.asarray(dense_b, np.float32)
    src_idx = np.asarray(src_idx).astype(np.int64)
    qry_idx = np.asarray(qry_idx).astype(np.int64)

    plans, bstart, bsz = _plan(qry_idx)
    NCH = max(len(p['chunks']) for p in plans)
    NCHP = ((NCH + 15) // 16) * 16
    NQ = NCHP * 16

    # filter regroup: G2[ax*32+c, t*64+o] = filt[az, ay, ax, c, o], t = az*4+ay
    G2 = np.zeros((128, 16 * 64), np.float32)
    for az in range(4):
        for ay in range(4):
            t = az * 4 + ay
            for ax in range(4):
                G2[ax * 32:(ax + 1) * 32, t * 64:(t + 1) * 64] = filt[az, ay, ax]

    dwb = dense_w.astype(bfloat16)
    dbb = dense_b.reshape(COUT, 1).astype(np.float32)
    in_maps = []
    for c, p in enumerate(plans):
        possrc, posqry, fsrc, qexp = _pack_core(p, bstart, pos, feats,
                                                qry_idx, src_idx, NCHP)
        ftT = np.zeros((CIN, NQ), bfloat16)
        ftT[:, 0:p['nq']] = feats[p['q0']:p['q0'] + p['nq']].T.astype(bfloat16)
        in_maps.append({
            "possrc": possrc, "posqry": posqry, "fsrc": fsrc, "qexp": qexp,
            "g2": G2, "featsT": ftT, "denw": dwb, "denb": dbb,
        })
    return NCHP, NQ, plans, in_maps


# ----------------------------------------------------------------------------
# Device kernel
# ----------------------------------------------------------------------------
def _build_bass(NCHP, NQ):
    import concourse.bass as bass
    import concourse.tile as tile
    from concourse import bacc, mybir
    from concourse.bass import AP

    f32 = mybir.dt.float32
    f32r = mybir.dt.float32r
    bf16 = mybir.dt.bfloat16
    i32 = mybir.dt.int32
    ALU = mybir.AluOpType
    ACT = mybir.ActivationFunctionType
    W = NCHP
    NGRP = W // 16
    SCL = float(2.0 / EXTENT) * 1.5  # tent scale: g = SCL*m + 1.5
    F4PI = float(4.0 / np.pi)

    nc = bacc.Bacc("TRN2", target_bir_lowering=False, debug=False)

    possrc = nc.dram_tensor("possrc", (128, W, 3), f32, kind="ExternalInput")
    posqry = nc.dram_tensor("posqry", (128, W, 3), f32, kind="ExternalInput")
    fsrc = nc.dram_tensor("fsrc", (128, W, CIN), bf16, kind="ExternalInput")
    qexp = nc.dram_tensor("qexp", (128, W * 256), bf16, kind="ExternalInput")
    g2 = nc.dram_tensor("g2", (128, 16 * 64), f32r, kind="ExternalInput")
    featsT = nc.dram_tensor("featsT", (CIN, NQ), bf16, kind="ExternalInput")
    denw = nc.dram_tensor("denw", (CIN, COUT), bf16, kind="ExternalInput")
    denb = nc.dram_tensor("denb", (COUT, 1), f32, kind="ExternalInput")

    outconvT = nc.dram_tensor("outconvT", (COUT, NQ), f32, kind="ExternalOutput")
    outdenseT = nc.dram_tensor("outdenseT", (COUT, NQ), f32, kind="ExternalOutput")

    with tile.TileContext(nc) as tc:
        with tc.tile_pool(name="inp", bufs=1) as inp, \
             tc.tile_pool(name="geo", bufs=1) as geo, \
             tc.tile_pool(name="tmp", bufs=1) as tmp, \
             tc.tile_pool(name="qex", bufs=6) as qex, \
             tc.tile_pool(name="lp", bufs=2) as lp, \
             tc.tile_pool(name="rp", bufs=2) as rp, \
             tc.tile_pool(name="atp", bufs=3) as atp, \
             tc.tile_pool(name="outp", bufs=2) as outp, \
             tc.tile_pool(name="ps1", bufs=3, space="PSUM") as ps1, \
             tc.tile_pool(name="ps2", bufs=2, space="PSUM") as ps2:

            # ---------------- input DMAs ----------------
            t_ps = inp.tile([128, W, 3], f32)
            t_pq = inp.tile([128, W, 3], f32)
            t_f = inp.tile([128, W, CIN], bf16)
            t_g2r = inp.tile([128, 16 * 64], f32r)
            t_ftT = inp.tile([CIN, NQ], bf16)
            t_dw = inp.tile([CIN, COUT], bf16)
            t_db = inp.tile([COUT, 1], f32)
            nc.sync.dma_start(t_ps[:], possrc[:])
            nc.sync.dma_start(t_pq[:], posqry[:])
            nc.sync.dma_start(t_f[:], fsrc[:])
            nc.sync.dma_start(t_g2r[:], g2[:])
            nc.sync.dma_start(t_ftT[:], featsT[:])
            nc.sync.dma_start(t_dw[:], denw[:])
            nc.sync.dma_start(t_db[:], denb[:])

            # ---------------- dense branch (overlaps geometry) ----------------
            for s_ in range((NQ + 511) // 512):
                j0 = s_ * 512
                j1 = min(NQ, j0 + 512)
                w_ = j1 - j0
                # dense shares the ps1 pool (tag s1) so ps2 holds only the
                # double-buffered tap accumulator (PSUM = 3*2 + 2 = 8 banks)
                pdt = ps1.tile([128, 1024], f32, space="PSUM",
                               name=f"pden_{s_}", tag="s1")
                pd = pdt[0:COUT, 0:512]
                nc.tensor.matmul(out=pd[:, 0:w_], lhsT=t_dw[:],
                                 rhs=t_ftT[:, j0:j1], start=True, stop=True)
                odt = outp.tile([COUT, 512], f32, tag="odst")
                nc.scalar.activation(odt[:, 0:w_], pd[:, 0:w_], ACT.Identity,
                                     bias=t_db[:, 0:1], scale=1.0)
                nc.sync.dma_start(outdenseT[:, j0:j1], odt[:, 0:w_])

            # iota constant: io12s[axis*4 + ax] = ax - 1.5
            io12i = tmp.tile([128, 12], i32)
            nc.gpsimd.iota(io12i[:], pattern=[[0, 3], [1, 4]], base=0,
                           channel_multiplier=0)
            io12f = tmp.tile([128, 12], f32)
            nc.vector.tensor_copy(io12f[:], io12i[:])
            io12s = geo.tile([128, 12], f32)
            nc.vector.tensor_scalar(io12s[:], io12f[:], -1.5, None, op0=ALU.add)

            # ---------------- geometry ----------------
            _tn = [0]
            _free_tags = []
            _tag_of = {}
            _seq = [0]

            def T(shape=(128, W), dt_=f32):
                if _free_tags:
                    tg = _free_tags.pop()
                else:
                    _tn[0] += 1
                    tg = f"t{_tn[0]}"
                _seq[0] += 1
                t = tmp.tile(list(shape), dt_, name=f"{tg}_u{_seq[0]}", tag=tg)
                _tag_of[id(t)] = tg
                return t

            def F(*ts):
                for t in ts:
                    _free_tags.append(_tag_of.pop(id(t)))

            TT = nc.vector.tensor_tensor
            TS = nc.vector.tensor_scalar
            STT = nc.vector.scalar_tensor_tensor
            ACTV = nc.scalar.activation
            RCP = nc.vector.reciprocal_approx_fast

            # Full-W geometry outputs consumed by the group loop
            w4 = geo.tile([128, W, 3, 4], bf16)
            zy = geo.tile([128, W, 16], bf16)

            def geom(lo, hw):
                """Geometry + tent weights for chunks [lo, lo+hw)."""
                def bc(t, n, stride=1):
                    return AP(t.tensor, t[:].offset,
                              [t[:].ap[0], [stride, hw], [0, n]])

                # d3 = ps - pq (unscaled; map is homogeneous, the 2/EXTENT
                # scale folds into the tent transform below)
                d3 = T((128, hw, 3))
                TT(out=d3[:], in0=t_ps[:, lo:lo + hw, :],
                   in1=t_pq[:, lo:lo + hw, :], op=ALU.subtract)
                z = d3[:, :, 2]
                sq3 = T((128, hw, 3))
                ACTV(sq3[:], d3[:], ACT.Square)
                xy2 = T((128, hw))
                TT(out=xy2[:], in0=sq3[:, :, 0], in1=sq3[:, :, 1], op=ALU.add)
                sq = T((128, hw))
                TT(out=sq[:], in0=xy2[:], in1=sq3[:, :, 2], op=ALU.add)
                norm = T((128, hw))
                ACTV(norm[:], sq[:], ACT.Sqrt)
                F(sq)
                nxy = T((128, hw))
                ACTV(nxy[:], xy2[:], ACT.Sqrt)
                pole = T((128, hw))
                STT(out=pole[:], in0=sq3[:, :, 2], scalar=1.25, in1=xy2[:],
                    op0=ALU.mult, op1=ALU.is_gt)
                F(xy2, sq3)

                azn = T((128, hw))
                ACTV(azn[:], z, ACT.Abs)
                den1 = T((128, hw))
                STT(out=den1[:], in0=azn[:], scalar=1e-13, in1=norm[:],
                    op0=ALU.add, op1=ALU.add)
                F(azn)
                rd1 = T((128, hw))
                RCP(rd1[:], den1[:])
                F(den1)
                t1a = T((128, hw))
                STT(out=t1a[:], in0=norm[:], scalar=3.0, in1=rd1[:],
                    op0=ALU.mult, op1=ALU.mult)
                F(rd1)
                s1 = T((128, hw))
                ACTV(s1[:], t1a[:], ACT.Sqrt)
                F(t1a)
                den2 = T((128, hw))
                TS(den2[:], nxy[:], 1e-13, None, op0=ALU.add)
                F(nxy)
                rd2 = T((128, hw))
                RCP(rd2[:], den2[:])
                F(den2)
                s2 = T((128, hw))
                TT(out=s2[:], in0=norm[:], in1=rd2[:], op=ALU.mult)
                F(rd2)
                dd = T((128, hw))
                TT(out=dd[:], in0=s1[:], in1=s2[:], op=ALU.subtract)
                F(s1)
                pw = T((128, hw))
                TT(out=pw[:], in0=pole[:], in1=dd[:], op=ALU.mult)
                F(dd)
                wq = T((128, hw))
                TT(out=wq[:], in0=s2[:], in1=pw[:], op=ALU.add)
                F(s2, pw)

                # m3 holds SCL * (cube coords): the tent scale is baked in
                # here so the d4 op below can be a plain TT (verifier caps
                # TS-class ops at 2 free dims).
                m3 = T((128, hw, 3))
                cyl2 = T((128, hw, 2))
                TT(out=cyl2[:], in0=d3[:, :, 0:2], in1=bc(wq, 2), op=ALU.mult)
                F(wq)
                # z: SCL*zc = 1.5*SCL*z + pole*(sign(z)*SCL*norm - 1.5*SCL*z)
                sgz = T((128, hw))
                ACTV(sgz[:], z, ACT.Sign)
                zcp = T((128, hw))
                STT(out=zcp[:], in0=sgz[:], scalar=SCL, in1=norm[:],
                    op0=ALU.mult, op1=ALU.mult)
                F(sgz, norm)
                u = T((128, hw))
                STT(out=u[:], in0=z, scalar=-1.5 * SCL, in1=zcp[:],
                    op0=ALU.mult, op1=ALU.add)
                F(zcp)
                pu = T((128, hw))
                TT(out=pu[:], in0=pole[:], in1=u[:], op=ALU.mult)
                F(u, pole)
                STT(out=m3[:, :, 2], in0=z, scalar=1.5 * SCL, in1=pu[:],
                    op0=ALU.mult, op1=ALU.add)
                F(pu, d3)

                # cylinder -> cube (x/y as [*, 2] pairs)
                sqc = T((128, hw, 2))
                ACTV(sqc[:], cyl2[:], ACT.Square)
                sqxy = T((128, hw))
                TT(out=sqxy[:], in0=sqc[:, :, 0], in1=sqc[:, :, 1], op=ALU.add)
                F(sqc)
                nrm = T((128, hw))
                ACTV(nrm[:], sqxy[:], ACT.Sqrt)
                F(sqxy)
                acl = T((128, hw, 2))
                ACTV(acl[:], cyl2[:], ACT.Abs)
                abr = T((128, hw))
                TT(out=abr[:], in0=acl[:, :, 1], in1=acl[:, :, 0], op=ALU.is_le)
                m2 = T((128, hw, 2))
                TS(m2[:], acl[:], 1e-12, None, op0=ALU.is_lt)
                F(acl)
                sf2 = T((128, hw, 2))
                TT(out=sf2[:], in0=cyl2[:], in1=m2[:], op=ALU.add)
                F(m2)
                inv2 = T((128, hw, 2))
                RCP(inv2[:], sf2[:])
                F(sf2)
                rat2 = T((128, hw, 2))
                TT(out=rat2[:], in0=cyl2[:],
                   in1=AP(inv2.tensor, inv2[:].offset + 1,
                          [inv2[:].ap[0], [2, hw], [-1, 2]]),
                   op=ALU.mult)
                F(inv2)
                at2v = T((128, hw, 2))
                ACTV(at2v[:], rat2[:], ACT.Arctan)
                F(rat2)
                sg2 = T((128, hw, 2))
                ACTV(sg2[:], cyl2[:], ACT.Sign)
                F(cyl2)
                t4 = T((128, hw, 4))
                # t4[0:2] = (tmpa, tmpb) = sign(xc,yc) * nrm * SCL
                STT(out=t4[:, :, 0:2], in0=sg2[:], scalar=SCL, in1=bc(nrm, 2),
                    op0=ALU.mult, op1=ALU.mult)
                F(sg2, nrm)
                # t4[2:4] = (xoe, yoe) = F4PI * arctan * (tmpb, tmpa)
                STT(out=t4[:, :, 2:4], in0=at2v[:], scalar=F4PI,
                    in1=AP(t4.tensor, t4[:].offset + 1,
                           [t4[:].ap[0], [4, hw], [-1, 2]]),
                    op0=ALU.mult, op1=ALU.mult)
                F(at2v)
                # del2 = (tmpa - xoe, yoe - tmpb)
                del2 = T((128, hw, 2))
                TT(out=del2[:],
                   in0=AP(t4.tensor, t4[:].offset + 0,
                          [t4[:].ap[0], [4, hw], [3, 2]]),
                   in1=AP(t4.tensor, t4[:].offset + 2,
                          [t4[:].ap[0], [4, hw], [-1, 2]]),
                   op=ALU.subtract)
                ad2 = T((128, hw, 2))
                TT(out=ad2[:], in0=del2[:], in1=bc(abr, 2), op=ALU.mult)
                F(del2, abr)
                # m3[:, :, 0:2] = (xoe, tmpb) + abr*delta
                TT(out=m3[:, :, 0:2],
                   in0=AP(t4.tensor, t4[:].offset + 2,
                          [t4[:].ap[0], [4, hw], [-1, 2]]),
                   in1=ad2[:], op=ALU.add)
                F(ad2, t4)

                # tent: d4[s, w, axis, ax] = SCL*m + 1.5 - ax
                d4 = T((128, hw, 3, 4))
                TT(out=d4[:],
                   in0=AP(m3.tensor, m3[:].offset,
                          [m3[:].ap[0], [3, hw], [1, 3], [0, 4]]),
                   in1=AP(io12s.tensor, io12s[:].offset,
                          [io12s[:].ap[0], [0, hw], [4, 3], [1, 4]]),
                   op=ALU.subtract)
                F(m3)
                a4 = T((128, hw, 3, 4))
                ACTV(a4[:], d4[:], ACT.Abs)
                F(d4)
                # w4 = relu(1 - |d4|)
                ACTV(AP(w4.tensor, w4[:].offset + lo * 12,
                        [w4[:].ap[0], [1, hw * 12]]),
                     AP(a4.tensor, a4[:].offset, [a4[:].ap[0], [1, hw * 12]]),
                     ACT.Relu, bias=1.0, scale=-1.0)
                F(a4)

                # zy[s, w, az, ay] = w4z (x) w4y   (bf16)
                TT(out=AP(zy.tensor, zy[:].offset + lo * 16,
                          [zy[:].ap[0], [16, hw], [4, 4], [1, 4]]),
                   in0=AP(w4.tensor, w4[:].offset + lo * 12 + 8,
                          [w4[:].ap[0], [12, hw], [1, 4], [0, 4]]),
                   in1=AP(w4.tensor, w4[:].offset + lo * 12 + 4,
                          [w4[:].ap[0], [12, hw], [0, 4], [1, 4]]),
                   op=ALU.mult)

            # ---------------- per-group: build L/R, stage-1, tap-GEMM --------
            # Software-pipelined: tap-GEMM matmuls of group g-1 are emitted
            # interleaved between the stage-1 quads of group g, so the PE
            # instruction stream never micro-idles (HAM stays un-throttled)
            # while the scalar engine drains PSUM->SBUF copies.
            at_tiles = {}
            po_tiles = {}

            def tap_quad(g, q):
                at_prev = at_tiles[g]
                if q == 0:
                    po_tiles[g] = ps2.tile([COUT, 256], f32, space="PSUM",
                                           name=f"po_{g}", tag="tap")
                po = po_tiles[g]
                for t in range(4 * q, 4 * q + 4):
                    rhs = AP(at_prev.tensor, at_prev[:].offset + t,
                             [at_prev[:].ap[0], [256, 16], [128, 2], [16, 8]])
                    nc.tensor.matmul(
                        out=po[:],
                        lhsT=t_g2r[:, t * 64:(t + 1) * 64],
                        rhs=rhs,
                        start=(t == 0), stop=(t == 15))
                if q == 3:
                    del at_tiles[g]
                    po = po_tiles.pop(g)
                    ost = outp.tile([COUT, 256], f32, name=f"ost_{g}",
                                    tag="ocst")
                    # alternate engines: ACT is the steady-state gate (93%
                    # busy from the A^T copies), DVE has ~13% slack
                    if g % 2 == 0:
                        nc.vector.tensor_copy(ost[:], po[:])
                    else:
                        nc.scalar.copy(ost[:], po[:])
                    nc.sync.dma_start(outconvT[:, g * 256:(g + 1) * 256],
                                      ost[:])

            def group_body(g):
                c0 = g * 16
                tq = qex.tile([128, 16 * 256], bf16, name=f"tq_{g}", tag="qex")
                nc.sync.dma_start(tq[:], qexp[:, g * 4096:(g + 1) * 4096])

                # R[s, ch, ax, c] = w4x (x) feats (bf16) — emitted before L:
                # R has no DMA dependency, so it absorbs the tail of the
                # group's qexp transfer
                R = rp.tile([128, 16 * 128], bf16, name=f"R_{g}", tag="R")
                TT(out=AP(R.tensor, R[:].offset,
                          [R[:].ap[0], [128, 16], [32, 4], [1, 32]]),
                   in0=AP(w4.tensor, w4[:].offset + c0 * 12,
                          [w4[:].ap[0], [12, 16], [1, 4], [0, 32]]),
                   in1=AP(t_f.tensor, t_f[:].offset + c0 * 32,
                          [t_f[:].ap[0], [32, 16], [0, 4], [1, 32]]),
                   op=ALU.mult)

                # L[s, ch, hq, t] = qexp * zy (packed bf16 2x mode)
                L = lp.tile([128, 16 * 256], bf16, name=f"L_{g}", tag="L")
                TT(out=AP(L.tensor, L[:].offset,
                          [L[:].ap[0], [256, 16], [16, 16], [1, 16]]),
                   in0=AP(tq.tensor, tq[:].offset,
                          [tq[:].ap[0], [256, 16], [16, 16], [1, 16]]),
                   in1=AP(zy.tensor, zy[:].offset + c0 * 16,
                          [zy[:].ap[0], [16, 16], [0, 16], [1, 16]]),
                   op=ALU.mult)

                at_st = atp.tile([128, 16 * 256], f32r, name=f"at_{g}", tag="at")
                at_tiles[g] = at_st
                for cl in range(0, 16, 4):
                    ps_t = ps1.tile([128, 1024], f32, space="PSUM",
                                    name=f"ps_{g}_{cl}", tag="s1")
                    for par in range(4):
                        ci = cl + par
                        nc.tensor.matmul(
                            out=ps_t[:, par * 256:(par + 1) * 256],
                            lhsT=R[:, ci * 128:(ci + 1) * 128],
                            rhs=L[:, ci * 256:(ci + 1) * 256],
                            start=True, stop=True)
                    # A^T copies ride the scalar engine; DVE builds next L/R
                    nc.scalar.copy(at_st[:, cl * 256:(cl + 4) * 256], ps_t[:])
                    if g >= 1:
                        tap_quad(g - 1, cl // 4)

            # single-shot geometry, then all groups (splitting geometry into
            # halves/quarters measured WORSE: ACT table reloads + extra per-op
            # overhead + mid-pipeline serialization outweigh the overlap win)
            geom(0, W)
            for g in range(NGRP):
                group_body(g)
            for q in range(4):
                tap_quad(NGRP - 1, q)

    nc.compile()
    return nc


# ----------------------------------------------------------------------------
# Entry point
# ----------------------------------------------------------------------------
def kernel(feats, pos, filt, dense_w, dense_b, src_idx, qry_idx):
    from concourse.bass_utils import run_bass_kernel_spmd

    NCHP, NQ, plans, in_maps = _prepare(feats, pos, filt, dense_w, dense_b,
                                        src_idx, qry_idx)
    key = (NCHP, NQ)
    if key not in _COMPILED:
        _COMPILED[key] = _build_bass(NCHP, NQ)
    nc = _COMPILED[key]

    res = run_bass_kernel_spmd(nc, in_maps, core_ids=list(range(NCORES)))

    ans_conv = np.zeros((N, COUT), np.float32)
    ans_dense = np.zeros((N, COUT), np.float32)
    for c, p in enumerate(plans):
        outT = res.results[c]["outconvT"]
        for ci, (bA, bB) in enumerate(p['chunks']):
            for half, b in enumerate((bA, bB)):
                if b is None:
                    continue
                cols = ci * 16 + half * 8
                ans_conv[8 * b:8 * b + 8] = outT[:, cols:cols + 8].T
        dT = res.results[c]["outdenseT"]
        ans_dense[p['q0']:p['q0'] + p['nq']] = dT[:, 0:p['nq']].T
    return ans_conv, ans_dense


# revision 39
# speedup vs baseline: 1.0069x; 1.0069x over previous
"""Trainium2 Bass kernel for nn_ContinuousConvolutionBlock (gnn_message_passing).

Strategy (per sharding hint: partition points across 8 cores; each core owns its
queries' scatter-reduce and tap-GEMM; filter + dense weights replicated):

Host side (index plumbing / input marshalling only - zero FLOPs):
  - qry_idx is sorted; queries are grouped into 8-query blocks, blocks paired
    into 128-edge-slot "chunks" (two-pointer bin packing, ~3% padding).
  - Per-core per-slot payload arrays (pos[src], pos[qry], feats[src] in bf16,
    and the expanded query one-hot qexp[slot, chunk, hq, t] in bf16) are
    marshalled on host and DMA'd in dense layouts.  qexp is pure indexing
    (0/1 one-hot replicated over the 16 tap-pairs) - uploading it lets the
    DVE build the L matmul operand at 2x packed-bf16 rate.

Device side (all FLOP-bearing compute):
  - Geometry: ball->cube volume-preserving map on unscaled deltas (the map is
    homogeneous; the 2/EXTENT scale folds into the corner transform), with
    x/y lanes processed as [*, 2] pairs and reciprocal_approx_fast.
  - Corner weights via the tent identity  w4[ax] = relu(1 - |g - ax|)
    (equivalent to the (1-f, f) one-hot pair incl. boundary clipping): one
    DVE op for d4 = 37.5*m + 1.5 - ax over all 3 axes, two ACT ops for
    abs + relu -> w4 [128, W, 3, 4] bf16.
  - Scatter-reduce as factored matmul per chunk (bf16): R[slot,(ax,c)] =
    w4x (x) feats, L[slot,(hq,t)] = qexp (.) (w4z (x) w4y  broadcast);
    PE computes A^T[(ax,c),(hq,t)] = R^T @ L per chunk into PSUM.
    L/R are built in one fused DVE op per 16-chunk group; L runs in the
    2x_1P packed-bf16 mode (all operands innermost step-1).
  - PSUM->SBUF copies of A^T run on the scalar engine (ACT), overlapping the
    DVE operand builds for the next group.
  - Tap-GEMM: out^T += G_t^T @ A^T-slices over the 16 tap-pairs t (f32r).
  - Dense branch: out_dense^T = dense_w^T @ feats^T (bf16) + bias via ACT.
  Outputs are produced transposed ([64, nq]); host transposes/reorders back.
"""
import sys
import os
sys.path.insert(0, '/opt/trn_rl_repo')
import numpy as np
from ml_dtypes import bfloat16

N = 30000
CIN = 32
COUT = 64
KS = 4
EXTENT = 0.08
NCORES = 8
NBLK = N // 8  # 3750 eight-query blocks

_COMPILED = {}

# Note: walrus --enable-ldw-opt=true was tried for the LDWEIGHTS-bound
# stage-1 (218ns/chunk vs 107ns matmul) but the resulting NEFF took the
# device down (NRT_EXEC_UNIT_UNRECOVERABLE); stock flags are kept.


# ----------------------------------------------------------------------------
# Host planning
# ----------------------------------------------------------------------------
def _plan(qry_idx):
    deg = np.bincount(qry_idx, minlength=N)
    bsz = deg.reshape(NBLK, 8).sum(1)
    bstart = np.concatenate([[0], np.cumsum(bsz)]).astype(np.int64)
    per = [NBLK // NCORES + (1 if c < NBLK % NCORES else 0) for c in range(NCORES)]
    b0 = np.concatenate([[0], np.cumsum(per)]).astype(np.int64)
    plans = []
    for c in range(NCORES):
        blocks = list(range(b0[c], b0[c + 1]))
        asc = sorted(blocks, key=lambda b: bsz[b])
        chunks = []
        lo, hi = 0, len(asc) - 1
        while lo <= hi:
            if lo == hi:
                chunks.append((asc[hi], None)); break
            if bsz[asc[hi]] + bsz[asc[lo]] <= 128:
                chunks.append((asc[hi], asc[lo])); hi -= 1; lo += 1
            else:
                chunks.append((asc[hi], None)); hi -= 1
        plans.append(dict(blocks=blocks, chunks=chunks, q0=int(8 * b0[c]),
                          nq=int(8 * (b0[c + 1] - b0[c]))))
    return plans, bstart, bsz


def _pack_core(plan_c, bstart, pos, feats, qry_idx, src_idx, NCHP):
    """Build per-slot payload arrays."""
    possrc = np.zeros((128, NCHP, 3), np.float32)
    posqry = np.zeros((128, NCHP, 3), np.float32)
    fsrc = np.zeros((128, NCHP, CIN), np.float32)
    qloc = np.full((128, NCHP), -1, np.int32)
    for ci, (bA, bB) in enumerate(plan_c['chunks']):
        s = 0
        for half, b in enumerate((bA, bB)):
            if b is None:
                continue
            e0, e1 = int(bstart[b]), int(bstart[b + 1])
            n = e1 - e0
            sl = slice(s, s + n)
            possrc[sl, ci, :] = pos[src_idx[e0:e1]]
            posqry[sl, ci, :] = pos[qry_idx[e0:e1]]
            fsrc[sl, ci, :] = feats[src_idx[e0:e1]]
            qloc[sl, ci] = (qry_idx[e0:e1] - 8 * b) + 8 * half
            s += n
    # expanded one-hot: qexp[s, w, hq, t] = (qloc[s,w] == hq), any t
    oh = (qloc[:, :, None] == np.arange(16, dtype=np.int32)[None, None, :])
    qexp = np.broadcast_to(oh[:, :, :, None], (128, NCHP, 16, 16))
    qexp = np.ascontiguousarray(qexp).astype(bfloat16).reshape(128, NCHP * 256)
    return possrc, posqry, fsrc.astype(bfloat16), qexp


def _prepare(feats, pos, filt, dense_w, dense_b, src_idx, qry_idx):
    """Plan + marshal all per-core input maps. Returns (NCHP, NQ, plans, in_maps)."""
    feats = np.ascontiguousarray(np.asarray(feats, np.float32))
    pos = np.ascontiguousarray(np.asarray(pos, np.float32))
    filt = np.asarray(filt, np.float32)
    dense_w = np.asarray(dense_w, np.float32)
    dense_b = np.asarray(dense_b, np.float32)
    src_idx = np.asarray(src_idx).astype(np.int64)
    qry_idx = np.asarray(qry_idx).astype(np.int64)

    plans, bstart, bsz = _plan(qry_idx)
    NCH = max(len(p['chunks']) for p in plans)
    NCHP = ((NCH + 15) // 16) * 16
    NQ = NCHP * 16

    # filter regroup: G2[ax*32+c, t*64+o] = filt[az, ay, ax, c, o], t = az*4+ay
    G2 = np.zeros((128, 16 * 64), np.float32)
    for az in range(4):
        for ay in range(4):
            t = az * 4 + ay
            for ax in range(4):
                G2[ax * 32:(ax + 1) * 32, t * 64:(t + 1) * 64] = filt[az, ay, ax]

    dwb = dense_w.astype(bfloat16)
    dbb = dense_b.reshape(COUT, 1).astype(np.float32)
    in_maps = []
    for c, p in enumerate(plans):
        possrc, posqry, fsrc, qexp = _pack_core(p, bstart, pos, feats,
                                                qry_idx, src_idx, NCHP)
        ftT = np.zeros((CIN, NQ), bfloat16)
        ftT[:, 0:p['nq']] = feats[p['q0']:p['q0'] + p['nq']].T.astype(bfloat16)
        in_maps.append({
            "possrc": possrc, "posqry": posqry, "fsrc": fsrc, "qexp": qexp,
            "g2": G2, "featsT": ftT, "denw": dwb, "denb": dbb,
        })
    return NCHP, NQ, plans, in_maps


# ----------------------------------------------------------------------------
# Device kernel
# ----------------------------------------------------------------------------
def _build_bass(NCHP, NQ):
    import concourse.bass as bass
    import concourse.tile as tile
    from concourse import bacc, mybir
    from concourse.bass import AP

    f32 = mybir.dt.float32
    f32r = mybir.dt.float32r
    bf16 = mybir.dt.bfloat16
    i32 = mybir.dt.int32
    ALU = mybir.AluOpType
    ACT = mybir.ActivationFunctionType
    W = NCHP
    NGRP = W // 16
    SCL = float(2.0 / EXTENT) * 1.5  # tent scale: g = SCL*m + 1.5
    F4PI = float(4.0 / np.pi)

    nc = bacc.Bacc("TRN2", target_bir_lowering=False, debug=False)

    possrc = nc.dram_tensor("possrc", (128, W, 3), f32, kind="ExternalInput")
    posqry = nc.dram_tensor("posqry", (128, W, 3), f32, kind="ExternalInput")
    fsrc = nc.dram_tensor("fsrc", (128, W, CIN), bf16, kind="ExternalInput")
    qexp = nc.dram_tensor("qexp", (128, W * 256), bf16, kind="ExternalInput")
    g2 = nc.dram_tensor("g2", (128, 16 * 64), f32r, kind="ExternalInput")
    featsT = nc.dram_tensor("featsT", (CIN, NQ), bf16, kind="ExternalInput")
    denw = nc.dram_tensor("denw", (CIN, COUT), bf16, kind="ExternalInput")
    denb = nc.dram_tensor("denb", (COUT, 1), f32, kind="ExternalInput")

    outconvT = nc.dram_tensor("outconvT", (COUT, NQ), f32, kind="ExternalOutput")
    outdenseT = nc.dram_tensor("outdenseT", (COUT, NQ), f32, kind="ExternalOutput")

    with tile.TileContext(nc) as tc:
        with tc.tile_pool(name="inp", bufs=1) as inp, \
             tc.tile_pool(name="geo", bufs=1) as geo, \
             tc.tile_pool(name="tmp", bufs=1) as tmp, \
             tc.tile_pool(name="qex", bufs=5) as qex, \
             tc.tile_pool(name="lp", bufs=3) as lp, \
             tc.tile_pool(name="rp", bufs=2) as rp, \
             tc.tile_pool(name="atp", bufs=3) as atp, \
             tc.tile_pool(name="outp", bufs=3) as outp, \
             tc.tile_pool(name="ps1", bufs=3, space="PSUM") as ps1, \
             tc.tile_pool(name="ps2", bufs=2, space="PSUM") as ps2:

            # ---------------- input DMAs ----------------
            t_ps = inp.tile([128, W, 3], f32)
            t_pq = inp.tile([128, W, 3], f32)
            t_f = inp.tile([128, W, CIN], bf16)
            t_g2r = inp.tile([128, 16 * 64], f32r)
            t_ftT = inp.tile([CIN, NQ], bf16)
            t_dw = inp.tile([CIN, COUT], bf16)
            t_db = inp.tile([COUT, 1], f32)
            nc.sync.dma_start(t_ps[:], possrc[:])
            nc.sync.dma_start(t_pq[:], posqry[:])
            nc.sync.dma_start(t_f[:], fsrc[:])
            nc.sync.dma_start(t_g2r[:], g2[:])
            nc.sync.dma_start(t_ftT[:], featsT[:])
            nc.sync.dma_start(t_dw[:], denw[:])
            nc.sync.dma_start(t_db[:], denb[:])

            # ---------------- dense branch (overlaps geometry) ----------------
            for s_ in range((NQ + 511) // 512):
                j0 = s_ * 512
                j1 = min(NQ, j0 + 512)
                w_ = j1 - j0
                # dense shares the ps1 pool (tag s1) so ps2 holds only the
                # double-buffered tap accumulator (PSUM = 3*2 + 2 = 8 banks)
                pdt = ps1.tile([128, 1024], f32, space="PSUM",
                               name=f"pden_{s_}", tag="s1")
                pd = pdt[0:COUT, 0:512]
                nc.tensor.matmul(out=pd[:, 0:w_], lhsT=t_dw[:],
                                 rhs=t_ftT[:, j0:j1], start=True, stop=True)
                odt = outp.tile([COUT, 512], f32, tag="odst")
                nc.scalar.activation(odt[:, 0:w_], pd[:, 0:w_], ACT.Identity,
                                     bias=t_db[:, 0:1], scale=1.0)
                nc.sync.dma_start(outdenseT[:, j0:j1], odt[:, 0:w_])

            # iota constant: io12s[axis*4 + ax] = ax - 1.5
            io12i = tmp.tile([128, 12], i32)
            nc.gpsimd.iota(io12i[:], pattern=[[0, 3], [1, 4]], base=0,
                           channel_multiplier=0)
            io12f = tmp.tile([128, 12], f32)
            nc.vector.tensor_copy(io12f[:], io12i[:])
            io12s = geo.tile([128, 12], f32)
            nc.vector.tensor_scalar(io12s[:], io12f[:], -1.5, None, op0=ALU.add)

            # ---------------- geometry ----------------
            _tn = [0]
            _free_tags = []
            _tag_of = {}
            _seq = [0]

            def T(shape=(128, W), dt_=f32):
                if _free_tags:
                    tg = _free_tags.pop()
                else:
                    _tn[0] += 1
                    tg = f"t{_tn[0]}"
                _seq[0] += 1
                t = tmp.tile(list(shape), dt_, name=f"{tg}_u{_seq[0]}", tag=tg)
                _tag_of[id(t)] = tg
                return t

            def F(*ts):
                for t in ts:
                    _free_tags.append(_tag_of.pop(id(t)))

            TT = nc.vector.tensor_tensor
            TS = nc.vector.tensor_scalar
            STT = nc.vector.scalar_tensor_tensor
            ACTV = nc.scalar.activation
            RCP = nc.vector.reciprocal_approx_fast

            # Full-W geometry outputs consumed by the group loop
            w4 = geo.tile([128, W, 3, 4], bf16)
            zy = geo.tile([128, W, 16], bf16)

            def geom(lo, hw):
                """Geometry + tent weights for chunks [lo, lo+hw)."""
                def bc(t, n, stride=1):
                    return AP(t.tensor, t[:].offset,
                              [t[:].ap[0], [stride, hw], [0, n]])

                # d3 = ps - pq (unscaled; map is homogeneous, the 2/EXTENT
                # scale folds into the tent transform below)
                d3 = T((128, hw, 3))
                TT(out=d3[:], in0=t_ps[:, lo:lo + hw, :],
                   in1=t_pq[:, lo:lo + hw, :], op=ALU.subtract)
                z = d3[:, :, 2]
                sq3 = T((128, hw, 3))
                ACTV(sq3[:], d3[:], ACT.Square)
                xy2 = T((128, hw))
                TT(out=xy2[:], in0=sq3[:, :, 0], in1=sq3[:, :, 1], op=ALU.add)
                sq = T((128, hw))
                TT(out=sq[:], in0=xy2[:], in1=sq3[:, :, 2], op=ALU.add)
                norm = T((128, hw))
                ACTV(norm[:], sq[:], ACT.Sqrt)
                F(sq)
                nxy = T((128, hw))
                ACTV(nxy[:], xy2[:], ACT.Sqrt)
                pole = T((128, hw))
                STT(out=pole[:], in0=sq3[:, :, 2], scalar=1.25, in1=xy2[:],
                    op0=ALU.mult, op1=ALU.is_gt)
                F(xy2, sq3)

                azn = T((128, hw))
                ACTV(azn[:], z, ACT.Abs)
                den1 = T((128, hw))
                STT(out=den1[:], in0=azn[:], scalar=1e-13, in1=norm[:],
                    op0=ALU.add, op1=ALU.add)
                F(azn)
                rd1 = T((128, hw))
                RCP(rd1[:], den1[:])
                F(den1)
                t1a = T((128, hw))
                STT(out=t1a[:], in0=norm[:], scalar=3.0, in1=rd1[:],
                    op0=ALU.mult, op1=ALU.mult)
                F(rd1)
                s1 = T((128, hw))
                ACTV(s1[:], t1a[:], ACT.Sqrt)
                F(t1a)
                den2 = T((128, hw))
                TS(den2[:], nxy[:], 1e-13, None, op0=ALU.add)
                F(nxy)
                rd2 = T((128, hw))
                RCP(rd2[:], den2[:])
                F(den2)
                s2 = T((128, hw))
                TT(out=s2[:], in0=norm[:], in1=rd2[:], op=ALU.mult)
                F(rd2)
                dd = T((128, hw))
                TT(out=dd[:], in0=s1[:], in1=s2[:], op=ALU.subtract)
                F(s1)
                pw = T((128, hw))
                TT(out=pw[:], in0=pole[:], in1=dd[:], op=ALU.mult)
                F(dd)
                wq = T((128, hw))
                TT(out=wq[:], in0=s2[:], in1=pw[:], op=ALU.add)
                F(s2, pw)

                # m3 holds SCL * (cube coords): the tent scale is baked in
                # here so the d4 op below can be a plain TT (verifier caps
                # TS-class ops at 2 free dims).
                m3 = T((128, hw, 3))
                cyl2 = T((128, hw, 2))
                TT(out=cyl2[:], in0=d3[:, :, 0:2], in1=bc(wq, 2), op=ALU.mult)
                F(wq)
                # z: SCL*zc = 1.5*SCL*z + pole*(sign(z)*SCL*norm - 1.5*SCL*z)
                sgz = T((128, hw))
                ACTV(sgz[:], z, ACT.Sign)
                zcp = T((128, hw))
                STT(out=zcp[:], in0=sgz[:], scalar=SCL, in1=norm[:],
                    op0=ALU.mult, op1=ALU.mult)
                F(sgz, norm)
                u = T((128, hw))
                STT(out=u[:], in0=z, scalar=-1.5 * SCL, in1=zcp[:],
                    op0=ALU.mult, op1=ALU.add)
                F(zcp)
                pu = T((128, hw))
                TT(out=pu[:], in0=pole[:], in1=u[:], op=ALU.mult)
                F(u, pole)
                STT(out=m3[:, :, 2], in0=z, scalar=1.5 * SCL, in1=pu[:],
                    op0=ALU.mult, op1=ALU.add)
                F(pu, d3)

                # cylinder -> cube (x/y as [*, 2] pairs)
                sqc = T((128, hw, 2))
                ACTV(sqc[:], cyl2[:], ACT.Square)
                sqxy = T((128, hw))
                TT(out=sqxy[:], in0=sqc[:, :, 0], in1=sqc[:, :, 1], op=ALU.add)
                F(sqc)
                nrm = T((128, hw))
                ACTV(nrm[:], sqxy[:], ACT.Sqrt)
                F(sqxy)
                acl = T((128, hw, 2))
                ACTV(acl[:], cyl2[:], ACT.Abs)
                abr = T((128, hw))
                TT(out=abr[:], in0=acl[:, :, 1], in1=acl[:, :, 0], op=ALU.is_le)
                m2 = T((128, hw, 2))
                TS(m2[:], acl[:], 1e-12, None, op0=ALU.is_lt)
                F(acl)
                sf2 = T((128, hw, 2))
                TT(out=sf2[:], in0=cyl2[:], in1=m2[:], op=ALU.add)
                F(m2)
                inv2 = T((128, hw, 2))
                RCP(inv2[:], sf2[:])
                F(sf2)
                rat2 = T((128, hw, 2))
                TT(out=rat2[:], in0=cyl2[:],
                   in1=AP(inv2.tensor, inv2[:].offset + 1,
                          [inv2[:].ap[0], [2, hw], [-1, 2]]),
                   op=ALU.mult)
                F(inv2)
                at2v = T((128, hw, 2))
                ACTV(at2v[:], rat2[:], ACT.Arctan)
                F(rat2)
                sg2 = T((128, hw, 2))
                ACTV(sg2[:], cyl2[:], ACT.Sign)
                F(cyl2)
                t4 = T((128, hw, 4))
                # t4[0:2] = (tmpa, tmpb) = sign(xc,yc) * nrm * SCL
                STT(out=t4[:, :, 0:2], in0=sg2[:], scalar=SCL, in1=bc(nrm, 2),
                    op0=ALU.mult, op1=ALU.mult)
                F(sg2, nrm)
                # t4[2:4] = (xoe, yoe) = F4PI * arctan * (tmpb, tmpa)
                STT(out=t4[:, :, 2:4], in0=at2v[:], scalar=F4PI,
                    in1=AP(t4.tensor, t4[:].offset + 1,
                           [t4[:].ap[0], [4, hw], [-1, 2]]),
                    op0=ALU.mult, op1=ALU.mult)
                F(at2v)
                # del2 = (tmpa - xoe, yoe - tmpb)
                del2 = T((128, hw, 2))
                TT(out=del2[:],
                   in0=AP(t4.tensor, t4[:].offset + 0,
                          [t4[:].ap[0], [4, hw], [3, 2]]),
                   in1=AP(t4.tensor, t4[:].offset + 2,
                          [t4[:].ap[0], [4, hw], [-1, 2]]),
                   op=ALU.subtract)
                ad2 = T((128, hw, 2))
                TT(out=ad2[:], in0=del2[:], in1=bc(abr, 2), op=ALU.mult)
                F(del2, abr)
                # m3[:, :, 0:2] = (xoe, tmpb) + abr*delta
                TT(out=m3[:, :, 0:2],
                   in0=AP(t4.tensor, t4[:].offset + 2,
                          [t4[:].ap[0], [4, hw], [-1, 2]]),
                   in1=ad2[:], op=ALU.add)
                F(ad2, t4)

                # tent: d4[s, w, axis, ax] = SCL*m + 1.5 - ax
                d4 = T((128, hw, 3, 4))
                TT(out=d4[:],
                   in0=AP(m3.tensor, m3[:].offset,
                          [m3[:].ap[0], [3, hw], [1, 3], [0, 4]]),
                   in1=AP(io12s.tensor, io12s[:].offset,
                          [io12s[:].ap[0], [0, hw], [4, 3], [1, 4]]),
                   op=ALU.subtract)
                F(m3)
                a4 = T((128, hw, 3, 4))
                ACTV(a4[:], d4[:], ACT.Abs)
                F(d4)
                # w4 = relu(1 - |d4|)
                ACTV(AP(w4.tensor, w4[:].offset + lo * 12,
                        [w4[:].ap[0], [1, hw * 12]]),
                     AP(a4.tensor, a4[:].offset, [a4[:].ap[0], [1, hw * 12]]),
                     ACT.Relu, bias=1.0, scale=-1.0)
                F(a4)

                # zy[s, w, az, ay] = w4z (x) w4y   (bf16)
                TT(out=AP(zy.tensor, zy[:].offset + lo * 16,
                          [zy[:].ap[0], [16, hw], [4, 4], [1, 4]]),
                   in0=AP(w4.tensor, w4[:].offset + lo * 12 + 8,
                          [w4[:].ap[0], [12, hw], [1, 4], [0, 4]]),
                   in1=AP(w4.tensor, w4[:].offset + lo * 12 + 4,
                          [w4[:].ap[0], [12, hw], [0, 4], [1, 4]]),
                   op=ALU.mult)

            # ---------------- per-group: build L/R, stage-1, tap-GEMM --------
            # Software-pipelined: tap-GEMM matmuls of group g-1 are emitted
            # interleaved between the stage-1 quads of group g, so the PE
            # instruction stream never micro-idles (HAM stays un-throttled)
            # while the scalar engine drains PSUM->SBUF copies.
            at_tiles = {}
            po_tiles = {}

            def tap_quad(g, q):
                at_prev = at_tiles[g]
                if q == 0:
                    po_tiles[g] = ps2.tile([COUT, 256], f32, space="PSUM",
                                           name=f"po_{g}", tag="tap")
                po = po_tiles[g]
                for t in range(4 * q, 4 * q + 4):
                    rhs = AP(at_prev.tensor, at_prev[:].offset + t,
                             [at_prev[:].ap[0], [256, 16], [128, 2], [16, 8]])
                    nc.tensor.matmul(
                        out=po[:],
                        lhsT=t_g2r[:, t * 64:(t + 1) * 64],
                        rhs=rhs,
                        start=(t == 0), stop=(t == 15))
                if q == 3:
                    del at_tiles[g]
                    po = po_tiles.pop(g)
                    ost = outp.tile([COUT, 256], f32, name=f"ost_{g}",
                                    tag="ocst")
                    # alternate engines: ACT is the steady-state gate (93%
                    # busy from the A^T copies), DVE has ~13% slack
                    if g % 2 == 0:
                        nc.vector.tensor_copy(ost[:], po[:])
                    else:
                        nc.scalar.copy(ost[:], po[:])
                    nc.sync.dma_start(outconvT[:, g * 256:(g + 1) * 256],
                                      ost[:])

            def group_body(g):
                c0 = g * 16
                tq = qex.tile([128, 16 * 256], bf16, name=f"tq_{g}", tag="qex")
                nc.sync.dma_start(tq[:], qexp[:, g * 4096:(g + 1) * 4096])

                # L[s, ch, hq, t] = qexp * zy (packed bf16 2x mode)
                L = lp.tile([128, 16 * 256], bf16, name=f"L_{g}", tag="L")
                TT(out=AP(L.tensor, L[:].offset,
                          [L[:].ap[0], [256, 16], [16, 16], [1, 16]]),
                   in0=AP(tq.tensor, tq[:].offset,
                          [tq[:].ap[0], [256, 16], [16, 16], [1, 16]]),
                   in1=AP(zy.tensor, zy[:].offset + c0 * 16,
                          [zy[:].ap[0], [16, 16], [0, 16], [1, 16]]),
                   op=ALU.mult)

                # R[s, ch, ax, c] = w4x (x) feats (bf16)
                R = rp.tile([128, 16 * 128], bf16, name=f"R_{g}", tag="R")
                TT(out=AP(R.tensor, R[:].offset,
                          [R[:].ap[0], [128, 16], [32, 4], [1, 32]]),
                   in0=AP(w4.tensor, w4[:].offset + c0 * 12,
                          [w4[:].ap[0], [12, 16], [1, 4], [0, 32]]),
                   in1=AP(t_f.tensor, t_f[:].offset + c0 * 32,
                          [t_f[:].ap[0], [32, 16], [0, 4], [1, 32]]),
                   op=ALU.mult)

                at_st = atp.tile([128, 16 * 256], f32r, name=f"at_{g}", tag="at")
                at_tiles[g] = at_st
                for cl in range(0, 16, 4):
                    ps_t = ps1.tile([128, 1024], f32, space="PSUM",
                                    name=f"ps_{g}_{cl}", tag="s1")
                    for par in range(4):
                        ci = cl + par
                        nc.tensor.matmul(
                            out=ps_t[:, par * 256:(par + 1) * 256],
                            lhsT=R[:, ci * 128:(ci + 1) * 128],
                            rhs=L[:, ci * 256:(ci + 1) * 256],
                            start=True, stop=True)
                    # A^T copies ride the scalar engine; DVE builds next L/R
                    nc.scalar.copy(at_st[:, cl * 256:(cl + 4) * 256], ps_t[:])
                    if g >= 1:
                        tap_quad(g - 1, cl // 4)

            # single-shot geometry, then all groups (splitting geometry into
            # halves/quarters measured WORSE: ACT table reloads + extra per-op
            # overhead + mid-pipeline serialization outweigh the overlap win)
            geom(0, W)
            for g in range(NGRP):
                group_body(g)
            for q in range(4):
                tap_quad(NGRP - 1, q)

    nc.compile()
    return nc


# ----------------------------------------------------------------------------
# Entry point
# ----------------------------------------------------------------------------
def kernel(feats, pos, filt, dense_w, dense_b, src_idx, qry_idx):
    from concourse.bass_utils import run_bass_kernel_spmd

    NCHP, NQ, plans, in_maps = _prepare(feats, pos, filt, dense_w, dense_b,
                                        src_idx, qry_idx)
    key = (NCHP, NQ)
    if key not in _COMPILED:
        _COMPILED[key] = _build_bass(NCHP, NQ)
    nc = _COMPILED[key]

    res = run_bass_kernel_spmd(nc, in_maps, core_ids=list(range(NCORES)))

    ans_conv = np.zeros((N, COUT), np.float32)
    ans_dense = np.zeros((N, COUT), np.float32)
    for c, p in enumerate(plans):
        outT = res.results[c]["outconvT"]
        for ci, (bA, bB) in enumerate(p['chunks']):
            for half, b in enumerate((bA, bB)):
                if b is None:
                    continue
                cols = ci * 16 + half * 8
                ans_conv[8 * b:8 * b + 8] = outT[:, cols:cols + 8].T
        dT = res.results[c]["outdenseT"]
        ans_dense[p['q0']:p['q0'] + p['nq']] = dT[:, 0:p['nq']].T
    return ans_conv, ans_dense


# revision 44
# speedup vs baseline: 1.0256x; 1.0186x over previous
"""Trainium2 Bass kernel for nn_ContinuousConvolutionBlock (gnn_message_passing).

Strategy (per sharding hint: partition points across 8 cores; each core owns its
queries' scatter-reduce and tap-GEMM; filter + dense weights replicated):

Host side (index plumbing / input marshalling only - zero FLOPs):
  - qry_idx is sorted; queries are grouped into 8-query blocks, blocks paired
    into 128-edge-slot "chunks" (two-pointer bin packing, ~3% padding).
  - Per-core per-slot payload arrays (pos[src], pos[qry], feats[src] in bf16,
    and the expanded query one-hot qexp[slot, chunk, hq, t] in bf16) are
    marshalled on host and DMA'd in dense layouts.  qexp is pure indexing
    (0/1 one-hot replicated over the 16 tap-pairs) - uploading it lets the
    DVE build the L matmul operand at 2x packed-bf16 rate.

Device side (all FLOP-bearing compute):
  - Geometry: ball->cube volume-preserving map on unscaled deltas (the map is
    homogeneous; the 2/EXTENT scale folds into the corner transform), with
    x/y lanes processed as [*, 2] pairs and reciprocal_approx_fast.
  - Corner weights via the tent identity  w4[ax] = relu(1 - |g - ax|)
    (equivalent to the (1-f, f) one-hot pair incl. boundary clipping): one
    DVE op for d4 = 37.5*m + 1.5 - ax over all 3 axes, two ACT ops for
    abs + relu -> w4 [128, W, 3, 4] bf16.
  - Scatter-reduce as factored matmul per chunk (bf16): R[slot,(ax,c)] =
    w4x (x) feats, L[slot,(hq,t)] = qexp (.) (w4z (x) w4y  broadcast);
    PE computes A^T[(ax,c),(hq,t)] = R^T @ L per chunk into PSUM.
    L/R are built in one fused DVE op per 16-chunk group; L runs in the
    2x_1P packed-bf16 mode (all operands innermost step-1).
  - PSUM->SBUF copies of A^T run on the scalar engine (ACT), overlapping the
    DVE operand builds for the next group.
  - Tap-GEMM: out^T += G_t^T @ A^T-slices over the 16 tap-pairs t (f32r).
  - Dense branch: out_dense^T = dense_w^T @ feats^T (bf16) + bias via ACT.
  Outputs are produced transposed ([64, nq]); host transposes/reorders back.
"""
import sys
import os
sys.path.insert(0, '/opt/trn_rl_repo')
import numpy as np
from ml_dtypes import bfloat16

N = 30000
CIN = 32
COUT = 64
KS = 4
EXTENT = 0.08
NCORES = 8
NBLK = N // 8  # 3750 eight-query blocks

_COMPILED = {}

# Note: walrus --enable-ldw-opt=true was tried for the LDWEIGHTS-bound
# stage-1 (218ns/chunk vs 107ns matmul) but the resulting NEFF took the
# device down (NRT_EXEC_UNIT_UNRECOVERABLE); stock flags are kept.


# ----------------------------------------------------------------------------
# Host planning
# ----------------------------------------------------------------------------
def _plan(qry_idx):
    deg = np.bincount(qry_idx, minlength=N)
    bsz = deg.reshape(NBLK, 8).sum(1)
    bstart = np.concatenate([[0], np.cumsum(bsz)]).astype(np.int64)
    per = [NBLK // NCORES + (1 if c < NBLK % NCORES else 0) for c in range(NCORES)]
    b0 = np.concatenate([[0], np.cumsum(per)]).astype(np.int64)
    plans = []
    for c in range(NCORES):
        blocks = list(range(b0[c], b0[c + 1]))
        asc = sorted(blocks, key=lambda b: bsz[b])
        chunks = []
        lo, hi = 0, len(asc) - 1
        while lo <= hi:
            if lo == hi:
                chunks.append((asc[hi], None)); break
            if bsz[asc[hi]] + bsz[asc[lo]] <= 128:
                chunks.append((asc[hi], asc[lo])); hi -= 1; lo += 1
            else:
                chunks.append((asc[hi], None)); hi -= 1
        plans.append(dict(blocks=blocks, chunks=chunks, q0=int(8 * b0[c]),
                          nq=int(8 * (b0[c + 1] - b0[c]))))
    return plans, bstart, bsz


def _pack_core(plan_c, bstart, pos, feats, qry_idx, src_idx, NCHP):
    """Build per-slot payload arrays."""
    possrc = np.zeros((128, NCHP, 3), np.float32)
    posqry = np.zeros((128, NCHP, 3), np.float32)
    fsrc = np.zeros((128, NCHP, CIN), np.float32)
    qloc = np.full((128, NCHP), -1, np.int32)
    for ci, (bA, bB) in enumerate(plan_c['chunks']):
        s = 0
        for half, b in enumerate((bA, bB)):
            if b is None:
                continue
            e0, e1 = int(bstart[b]), int(bstart[b + 1])
            n = e1 - e0
            sl = slice(s, s + n)
            possrc[sl, ci, :] = pos[src_idx[e0:e1]]
            posqry[sl, ci, :] = pos[qry_idx[e0:e1]]
            fsrc[sl, ci, :] = feats[src_idx[e0:e1]]
            qloc[sl, ci] = (qry_idx[e0:e1] - 8 * b) + 8 * half
            s += n
    # expanded one-hot: qexp[s, w, hq, t] = (qloc[s,w] == hq), any t
    oh = (qloc[:, :, None] == np.arange(16, dtype=np.int32)[None, None, :])
    qexp = np.broadcast_to(oh[:, :, :, None], (128, NCHP, 16, 16))
    qexp = np.ascontiguousarray(qexp).astype(bfloat16).reshape(128, NCHP * 256)
    return possrc, posqry, fsrc.astype(bfloat16), qexp


def _prepare(feats, pos, filt, dense_w, dense_b, src_idx, qry_idx):
    """Plan + marshal all per-core input maps. Returns (NCHP, NQ, plans, in_maps)."""
    feats = np.ascontiguousarray(np.asarray(feats, np.float32))
    pos = np.ascontiguousarray(np.asarray(pos, np.float32))
    filt = np.asarray(filt, np.float32)
    dense_w = np.asarray(dense_w, np.float32)
    dense_b = np.asarray(dense_b, np.float32)
    src_idx = np.asarray(src_idx).astype(np.int64)
    qry_idx = np.asarray(qry_idx).astype(np.int64)

    plans, bstart, bsz = _plan(qry_idx)
    NCH = max(len(p['chunks']) for p in plans)
    NCHP = ((NCH + 15) // 16) * 16
    NQ = NCHP * 16

    # filter regroup: G2[ax*32+c, t*64+o] = filt[az, ay, ax, c, o], t = az*4+ay
    G2 = np.zeros((128, 16 * 64), np.float32)
    for az in range(4):
        for ay in range(4):
            t = az * 4 + ay
            for ax in range(4):
                G2[ax * 32:(ax + 1) * 32, t * 64:(t + 1) * 64] = filt[az, ay, ax]

    dwb = dense_w.astype(bfloat16)
    dbb = dense_b.reshape(COUT, 1).astype(np.float32)
    in_maps = []
    for c, p in enumerate(plans):
        possrc, posqry, fsrc, qexp = _pack_core(p, bstart, pos, feats,
                                                qry_idx, src_idx, NCHP)
        ftT = np.zeros((CIN, NQ), bfloat16)
        ftT[:, 0:p['nq']] = feats[p['q0']:p['q0'] + p['nq']].T.astype(bfloat16)
        in_maps.append({
            "possrc": possrc, "posqry": posqry, "fsrc": fsrc, "qexp": qexp,
            "g2": G2, "featsT": ftT, "denw": dwb, "denb": dbb,
        })
    return NCHP, NQ, plans, in_maps


# ----------------------------------------------------------------------------
# Device kernel
# ----------------------------------------------------------------------------
def _build_bass(NCHP, NQ):
    import concourse.bass as bass
    import concourse.tile as tile
    from concourse import bacc, mybir
    from concourse.bass import AP

    f32 = mybir.dt.float32
    f32r = mybir.dt.float32r
    bf16 = mybir.dt.bfloat16
    i32 = mybir.dt.int32
    ALU = mybir.AluOpType
    ACT = mybir.ActivationFunctionType
    W = NCHP
    NGRP = W // 16
    SCL = float(2.0 / EXTENT) * 1.5  # tent scale: g = SCL*m + 1.5
    F4PI = float(4.0 / np.pi)

    nc = bacc.Bacc("TRN2", target_bir_lowering=False, debug=False)

    possrc = nc.dram_tensor("possrc", (128, W, 3), f32, kind="ExternalInput")
    posqry = nc.dram_tensor("posqry", (128, W, 3), f32, kind="ExternalInput")
    fsrc = nc.dram_tensor("fsrc", (128, W, CIN), bf16, kind="ExternalInput")
    qexp = nc.dram_tensor("qexp", (128, W * 256), bf16, kind="ExternalInput")
    g2 = nc.dram_tensor("g2", (128, 16 * 64), f32r, kind="ExternalInput")
    featsT = nc.dram_tensor("featsT", (CIN, NQ), bf16, kind="ExternalInput")
    denw = nc.dram_tensor("denw", (CIN, COUT), bf16, kind="ExternalInput")
    denb = nc.dram_tensor("denb", (COUT, 1), f32, kind="ExternalInput")

    outconvT = nc.dram_tensor("outconvT", (COUT, NQ), f32, kind="ExternalOutput")
    outdenseT = nc.dram_tensor("outdenseT", (COUT, NQ), f32, kind="ExternalOutput")

    with tile.TileContext(nc) as tc:
        with tc.tile_pool(name="inp", bufs=1) as inp, \
             tc.tile_pool(name="geo", bufs=1) as geo, \
             tc.tile_pool(name="tmp", bufs=1) as tmp, \
             tc.tile_pool(name="qex", bufs=6) as qex, \
             tc.tile_pool(name="lp", bufs=2) as lp, \
             tc.tile_pool(name="rp", bufs=2) as rp, \
             tc.tile_pool(name="atp", bufs=3) as atp, \
             tc.tile_pool(name="outp", bufs=3) as outp, \
             tc.tile_pool(name="ps1", bufs=3, space="PSUM") as ps1, \
             tc.tile_pool(name="ps2", bufs=2, space="PSUM") as ps2:

            # ---------------- input DMAs ----------------
            t_ps = inp.tile([128, W, 3], f32)
            t_pq = inp.tile([128, W, 3], f32)
            t_f = inp.tile([128, W, CIN], bf16)
            t_g2r = inp.tile([128, 16 * 64], f32r)
            t_ftT = inp.tile([CIN, NQ], bf16)
            t_dw = inp.tile([CIN, COUT], bf16)
            t_db = inp.tile([COUT, 1], f32)
            nc.sync.dma_start(t_ps[:], possrc[:])
            nc.sync.dma_start(t_pq[:], posqry[:])
            nc.sync.dma_start(t_f[:], fsrc[:])
            nc.sync.dma_start(t_g2r[:], g2[:])
            nc.sync.dma_start(t_ftT[:], featsT[:])
            nc.sync.dma_start(t_dw[:], denw[:])
            nc.sync.dma_start(t_db[:], denb[:])

            # ---------------- dense branch (overlaps geometry) ----------------
            for s_ in range((NQ + 511) // 512):
                j0 = s_ * 512
                j1 = min(NQ, j0 + 512)
                w_ = j1 - j0
                # dense shares the ps1 pool (tag s1) so ps2 holds only the
                # double-buffered tap accumulator (PSUM = 3*2 + 2 = 8 banks)
                pdt = ps1.tile([128, 1024], f32, space="PSUM",
                               name=f"pden_{s_}", tag="s1")
                pd = pdt[0:COUT, 0:512]
                nc.tensor.matmul(out=pd[:, 0:w_], lhsT=t_dw[:],
                                 rhs=t_ftT[:, j0:j1], start=True, stop=True)
                odt = outp.tile([COUT, 512], f32, tag="odst")
                nc.scalar.activation(odt[:, 0:w_], pd[:, 0:w_], ACT.Identity,
                                     bias=t_db[:, 0:1], scale=1.0)
                nc.sync.dma_start(outdenseT[:, j0:j1], odt[:, 0:w_])

            # iota constant: io12s[axis*4 + ax] = ax - 1.5
            io12i = tmp.tile([128, 12], i32)
            nc.gpsimd.iota(io12i[:], pattern=[[0, 3], [1, 4]], base=0,
                           channel_multiplier=0)
            io12f = tmp.tile([128, 12], f32)
            nc.vector.tensor_copy(io12f[:], io12i[:])
            io12s = geo.tile([128, 12], f32)
            nc.vector.tensor_scalar(io12s[:], io12f[:], -1.5, None, op0=ALU.add)

            # ---------------- geometry ----------------
            _tn = [0]
            _free_tags = []
            _tag_of = {}
            _seq = [0]

            def T(shape=(128, W), dt_=f32):
                if _free_tags:
                    tg = _free_tags.pop()
                else:
                    _tn[0] += 1
                    tg = f"t{_tn[0]}"
                _seq[0] += 1
                t = tmp.tile(list(shape), dt_, name=f"{tg}_u{_seq[0]}", tag=tg)
                _tag_of[id(t)] = tg
                return t

            def F(*ts):
                for t in ts:
                    _free_tags.append(_tag_of.pop(id(t)))

            TT = nc.vector.tensor_tensor
            TS = nc.vector.tensor_scalar
            STT = nc.vector.scalar_tensor_tensor
            ACTV = nc.scalar.activation
            RCP = nc.vector.reciprocal_approx_fast

            # Full-W geometry outputs consumed by the group loop
            w4 = geo.tile([128, W, 3, 4], bf16)
            zy = geo.tile([128, W, 16], bf16)

            def geom(lo, hw):
                """Geometry + tent weights for chunks [lo, lo+hw)."""
                def bc(t, n, stride=1):
                    return AP(t.tensor, t[:].offset,
                              [t[:].ap[0], [stride, hw], [0, n]])

                # d3 = ps - pq (unscaled; map is homogeneous, the 2/EXTENT
                # scale folds into the tent transform below)
                d3 = T((128, hw, 3))
                TT(out=d3[:], in0=t_ps[:, lo:lo + hw, :],
                   in1=t_pq[:, lo:lo + hw, :], op=ALU.subtract)
                z = d3[:, :, 2]
                sq3 = T((128, hw, 3))
                ACTV(sq3[:], d3[:], ACT.Square)
                xy2 = T((128, hw))
                TT(out=xy2[:], in0=sq3[:, :, 0], in1=sq3[:, :, 1], op=ALU.add)
                sq = T((128, hw))
                TT(out=sq[:], in0=xy2[:], in1=sq3[:, :, 2], op=ALU.add)
                norm = T((128, hw))
                ACTV(norm[:], sq[:], ACT.Sqrt)
                F(sq)
                nxy = T((128, hw))
                ACTV(nxy[:], xy2[:], ACT.Sqrt)
                pole = T((128, hw))
                STT(out=pole[:], in0=sq3[:, :, 2], scalar=1.25, in1=xy2[:],
                    op0=ALU.mult, op1=ALU.is_gt)
                F(xy2, sq3)

                azn = T((128, hw))
                ACTV(azn[:], z, ACT.Abs)
                den1 = T((128, hw))
                STT(out=den1[:], in0=azn[:], scalar=1e-13, in1=norm[:],
                    op0=ALU.add, op1=ALU.add)
                F(azn)
                rd1 = T((128, hw))
                RCP(rd1[:], den1[:])
                F(den1)
                t1a = T((128, hw))
                STT(out=t1a[:], in0=norm[:], scalar=3.0, in1=rd1[:],
                    op0=ALU.mult, op1=ALU.mult)
                F(rd1)
                s1 = T((128, hw))
                ACTV(s1[:], t1a[:], ACT.Sqrt)
                F(t1a)
                den2 = T((128, hw))
                TS(den2[:], nxy[:], 1e-13, None, op0=ALU.add)
                F(nxy)
                rd2 = T((128, hw))
                RCP(rd2[:], den2[:])
                F(den2)
                s2 = T((128, hw))
                TT(out=s2[:], in0=norm[:], in1=rd2[:], op=ALU.mult)
                F(rd2)
                dd = T((128, hw))
                TT(out=dd[:], in0=s1[:], in1=s2[:], op=ALU.subtract)
                F(s1)
                pw = T((128, hw))
                TT(out=pw[:], in0=pole[:], in1=dd[:], op=ALU.mult)
                F(dd)
                wq = T((128, hw))
                TT(out=wq[:], in0=s2[:], in1=pw[:], op=ALU.add)
                F(s2, pw)

                # m3 holds SCL * (cube coords): the tent scale is baked in
                # here so the d4 op below can be a plain TT (verifier caps
                # TS-class ops at 2 free dims).
                m3 = T((128, hw, 3))
                cyl2 = T((128, hw, 2))
                TT(out=cyl2[:], in0=d3[:, :, 0:2], in1=bc(wq, 2), op=ALU.mult)
                F(wq)
                # z: SCL*zc = 1.5*SCL*z + pole*(sign(z)*SCL*norm - 1.5*SCL*z)
                sgz = T((128, hw))
                ACTV(sgz[:], z, ACT.Sign)
                zcp = T((128, hw))
                STT(out=zcp[:], in0=sgz[:], scalar=SCL, in1=norm[:],
                    op0=ALU.mult, op1=ALU.mult)
                F(sgz, norm)
                u = T((128, hw))
                STT(out=u[:], in0=z, scalar=-1.5 * SCL, in1=zcp[:],
                    op0=ALU.mult, op1=ALU.add)
                F(zcp)
                pu = T((128, hw))
                TT(out=pu[:], in0=pole[:], in1=u[:], op=ALU.mult)
                F(u, pole)
                STT(out=m3[:, :, 2], in0=z, scalar=1.5 * SCL, in1=pu[:],
                    op0=ALU.mult, op1=ALU.add)
                F(pu, d3)

                # cylinder -> cube (x/y as [*, 2] pairs)
                sqc = T((128, hw, 2))
                ACTV(sqc[:], cyl2[:], ACT.Square)
                sqxy = T((128, hw))
                TT(out=sqxy[:], in0=sqc[:, :, 0], in1=sqc[:, :, 1], op=ALU.add)
                F(sqc)
                nrm = T((128, hw))
                ACTV(nrm[:], sqxy[:], ACT.Sqrt)
                F(sqxy)
                acl = T((128, hw, 2))
                ACTV(acl[:], cyl2[:], ACT.Abs)
                abr = T((128, hw))
                TT(out=abr[:], in0=acl[:, :, 1], in1=acl[:, :, 0], op=ALU.is_le)
                m2 = T((128, hw, 2))
                TS(m2[:], acl[:], 1e-12, None, op0=ALU.is_lt)
                F(acl)
                sf2 = T((128, hw, 2))
                TT(out=sf2[:], in0=cyl2[:], in1=m2[:], op=ALU.add)
                F(m2)
                inv2 = T((128, hw, 2))
                RCP(inv2[:], sf2[:])
                F(sf2)
                rat2 = T((128, hw, 2))
                TT(out=rat2[:], in0=cyl2[:],
                   in1=AP(inv2.tensor, inv2[:].offset + 1,
                          [inv2[:].ap[0], [2, hw], [-1, 2]]),
                   op=ALU.mult)
                F(inv2)
                at2v = T((128, hw, 2))
                ACTV(at2v[:], rat2[:], ACT.Arctan)
                F(rat2)
                sg2 = T((128, hw, 2))
                ACTV(sg2[:], cyl2[:], ACT.Sign)
                F(cyl2)
                t4 = T((128, hw, 4))
                # t4[0:2] = (tmpa, tmpb) = sign(xc,yc) * nrm * SCL
                STT(out=t4[:, :, 0:2], in0=sg2[:], scalar=SCL, in1=bc(nrm, 2),
                    op0=ALU.mult, op1=ALU.mult)
                F(sg2, nrm)
                # t4[2:4] = (xoe, yoe) = F4PI * arctan * (tmpb, tmpa)
                STT(out=t4[:, :, 2:4], in0=at2v[:], scalar=F4PI,
                    in1=AP(t4.tensor, t4[:].offset + 1,
                           [t4[:].ap[0], [4, hw], [-1, 2]]),
                    op0=ALU.mult, op1=ALU.mult)
                F(at2v)
                # del2 = (tmpa - xoe, yoe - tmpb)
                del2 = T((128, hw, 2))
                TT(out=del2[:],
                   in0=AP(t4.tensor, t4[:].offset + 0,
                          [t4[:].ap[0], [4, hw], [3, 2]]),
                   in1=AP(t4.tensor, t4[:].offset + 2,
                          [t4[:].ap[0], [4, hw], [-1, 2]]),
                   op=ALU.subtract)
                ad2 = T((128, hw, 2))
                TT(out=ad2[:], in0=del2[:], in1=bc(abr, 2), op=ALU.mult)
                F(del2, abr)
                # m3[:, :, 0:2] = (xoe, tmpb) + abr*delta
                TT(out=m3[:, :, 0:2],
                   in0=AP(t4.tensor, t4[:].offset + 2,
                          [t4[:].ap[0], [4, hw], [-1, 2]]),
                   in1=ad2[:], op=ALU.add)
                F(ad2, t4)

                return m3

            def tent_half(m3, mlo, lo, hw):
                """d4 + tent for chunks [lo, lo+hw); m3 indexed from mlo."""
                d4 = T((128, hw, 3, 4))
                TT(out=d4[:],
                   in0=AP(m3.tensor, m3[:].offset + (lo - mlo) * 3,
                          [m3[:].ap[0], [3, hw], [1, 3], [0, 4]]),
                   in1=AP(io12s.tensor, io12s[:].offset,
                          [io12s[:].ap[0], [0, hw], [4, 3], [1, 4]]),
                   op=ALU.subtract)
                a4 = T((128, hw, 3, 4))
                ACTV(a4[:], d4[:], ACT.Abs)
                F(d4)
                # w4 = relu(1 - |d4|)
                ACTV(AP(w4.tensor, w4[:].offset + lo * 12,
                        [w4[:].ap[0], [1, hw * 12]]),
                     AP(a4.tensor, a4[:].offset, [a4[:].ap[0], [1, hw * 12]]),
                     ACT.Relu, bias=1.0, scale=-1.0)
                F(a4)

            def zy_half(lo, hw):
                # zy[s, w, az, ay] = w4z (x) w4y   (bf16)
                TT(out=AP(zy.tensor, zy[:].offset + lo * 16,
                          [zy[:].ap[0], [16, hw], [4, 4], [1, 4]]),
                   in0=AP(w4.tensor, w4[:].offset + lo * 12 + 8,
                          [w4[:].ap[0], [12, hw], [1, 4], [0, 4]]),
                   in1=AP(w4.tensor, w4[:].offset + lo * 12 + 4,
                          [w4[:].ap[0], [12, hw], [0, 4], [1, 4]]),
                   op=ALU.mult)

            # ---------------- per-group: build L/R, stage-1, tap-GEMM --------
            # Software-pipelined: tap-GEMM matmuls of group g-1 are emitted
            # interleaved between the stage-1 quads of group g, so the PE
            # instruction stream never micro-idles (HAM stays un-throttled)
            # while the scalar engine drains PSUM->SBUF copies.
            at_tiles = {}
            po_tiles = {}

            def tap_quad(g, q):
                at_prev = at_tiles[g]
                if q == 0:
                    po_tiles[g] = ps2.tile([COUT, 256], f32, space="PSUM",
                                           name=f"po_{g}", tag="tap")
                po = po_tiles[g]
                for t in range(4 * q, 4 * q + 4):
                    rhs = AP(at_prev.tensor, at_prev[:].offset + t,
                             [at_prev[:].ap[0], [256, 16], [128, 2], [16, 8]])
                    nc.tensor.matmul(
                        out=po[:],
                        lhsT=t_g2r[:, t * 64:(t + 1) * 64],
                        rhs=rhs,
                        start=(t == 0), stop=(t == 15))
                if q == 3:
                    del at_tiles[g]
                    po = po_tiles.pop(g)
                    ost = outp.tile([COUT, 256], f32, name=f"ost_{g}",
                                    tag="ocst")
                    # alternate engines: ACT is the steady-state gate (93%
                    # busy from the A^T copies), DVE has ~13% slack
                    if g % 2 == 0:
                        nc.vector.tensor_copy(ost[:], po[:])
                    else:
                        nc.scalar.copy(ost[:], po[:])
                    nc.sync.dma_start(outconvT[:, g * 256:(g + 1) * 256],
                                      ost[:])

            def group_body(g):
                c0 = g * 16
                tq = qex.tile([128, 16 * 256], bf16, name=f"tq_{g}", tag="qex")
                nc.sync.dma_start(tq[:], qexp[:, g * 4096:(g + 1) * 4096])

                # L[s, ch, hq, t] = qexp * zy (packed bf16 2x mode)
                L = lp.tile([128, 16 * 256], bf16, name=f"L_{g}", tag="L")
                TT(out=AP(L.tensor, L[:].offset,
                          [L[:].ap[0], [256, 16], [16, 16], [1, 16]]),
                   in0=AP(tq.tensor, tq[:].offset,
                          [tq[:].ap[0], [256, 16], [16, 16], [1, 16]]),
                   in1=AP(zy.tensor, zy[:].offset + c0 * 16,
                          [zy[:].ap[0], [16, 16], [0, 16], [1, 16]]),
                   op=ALU.mult)

                # R[s, ch, ax, c] = w4x (x) feats (bf16)
                R = rp.tile([128, 16 * 128], bf16, name=f"R_{g}", tag="R")
                TT(out=AP(R.tensor, R[:].offset,
                          [R[:].ap[0], [128, 16], [32, 4], [1, 32]]),
                   in0=AP(w4.tensor, w4[:].offset + c0 * 12,
                          [w4[:].ap[0], [12, 16], [1, 4], [0, 32]]),
                   in1=AP(t_f.tensor, t_f[:].offset + c0 * 32,
                          [t_f[:].ap[0], [32, 16], [0, 4], [1, 32]]),
                   op=ALU.mult)

                at_st = atp.tile([128, 16 * 256], f32r, name=f"at_{g}", tag="at")
                at_tiles[g] = at_st
                for cl in range(0, 16, 4):
                    ps_t = ps1.tile([128, 1024], f32, space="PSUM",
                                    name=f"ps_{g}_{cl}", tag="s1")
                    for par in range(4):
                        ci = cl + par
                        nc.tensor.matmul(
                            out=ps_t[:, par * 256:(par + 1) * 256],
                            lhsT=R[:, ci * 128:(ci + 1) * 128],
                            rhs=L[:, ci * 256:(ci + 1) * 256],
                            start=True, stop=True)
                    # A^T copies ride the scalar engine; DVE builds next L/R
                    nc.scalar.copy(at_st[:, cl * 256:(cl + 4) * 256], ps_t[:])
                    if g >= 1:
                        tap_quad(g - 1, cl // 4)

            # single-shot geometry (splitting the MAIN geometry into
            # halves/quarters measured WORSE: ACT table reloads + extra per-op
            # overhead), but the serial tail d4->abs->relu->zy is pipelined in
            # halves so the first groups start ~6us earlier: zy(half a) only
            # needs relu(half a), and the second half's ACT ops overlap the
            # first half's zy + L/R builds.
            m3 = geom(0, W)
            hh = (NGRP // 2) * 16
            tent_half(m3, 0, 0, hh)
            tent_half(m3, 0, hh, W - hh)
            F(m3)
            zy_half(0, hh)
            for g in range(NGRP // 2):
                group_body(g)
            zy_half(hh, W - hh)
            for g in range(NGRP // 2, NGRP):
                group_body(g)
            for q in range(4):
                tap_quad(NGRP - 1, q)

    nc.compile()
    return nc


# ----------------------------------------------------------------------------
# Entry point
# ----------------------------------------------------------------------------
def kernel(feats, pos, filt, dense_w, dense_b, src_idx, qry_idx):
    from concourse.bass_utils import run_bass_kernel_spmd

    NCHP, NQ, plans, in_maps = _prepare(feats, pos, filt, dense_w, dense_b,
                                        src_idx, qry_idx)
    key = (NCHP, NQ)
    if key not in _COMPILED:
        _COMPILED[key] = _build_bass(NCHP, NQ)
    nc = _COMPILED[key]

    res = run_bass_kernel_spmd(nc, in_maps, core_ids=list(range(NCORES)))

    ans_conv = np.zeros((N, COUT), np.float32)
    ans_dense = np.zeros((N, COUT), np.float32)
    for c, p in enumerate(plans):
        outT = res.results[c]["outconvT"]
        for ci, (bA, bB) in enumerate(p['chunks']):
            for half, b in enumerate((bA, bB)):
                if b is None:
                    continue
                cols = ci * 16 + half * 8
                ans_conv[8 * b:8 * b + 8] = outT[:, cols:cols + 8].T
        dT = res.results[c]["outdenseT"]
        ans_dense[p['q0']:p['q0'] + p['nq']] = dT[:, 0:p['nq']].T
    return ans_conv, ans_dense
